# revision 5
# baseline (speedup 1.0000x reference)
"""Trainium2 Bass kernel for 3x3 (k=2m+1) morphological erosion (sliding-window
min) over [B, C, H, W] fp32, B=8 sharded across 8 NeuronCores (one batch per
core).

Numerics: min commutes with monotone rounding, so the device pipeline runs in
bf16 end-to-end (host converts fp32->bf16 on the way in, bf16->fp32 on the way
out). The device output equals bf16(exact fp32 min): max rel err 2^-9 ~ 0.2%,
well inside the 2e-2 gate. bf16 halves HBM traffic (32 MB/core/pass) and
doubles DVE tensor_tensor throughput (2x_1P packed mode).

Scheme (per core, shard = one batch of C=8 channel images, 1024x1024):
  - each partition holds RPP=8 consecutive image rows in its free dim; the
    vertical (row) min is a free-dim shifted tensor_tensor except at the 2
    per-partition boundary rows, whose missing neighbor rows are staged into
    a side tile "bt" via partition-shifted SBUF->SBUF DMA on the Pool SWDGE
    (DMA has no partition-alignment restriction; compute engines require
    start partition 0/32/64/96).
  - loads carry NO column halo: each partition's block is one fully aligned
    contiguous 16 KiB DRAM segment. (An earlier variant memset PAD halo
    columns beside the DMA'd data; sub-32B-beat writes from different queues
    RMW-race on hardware and corrupted ~1e-7 of elements at the borders.)
  - bf16 2x_1P packing requires step +-1 AND 4-byte-aligned operands. All
    V-pass operands shift by whole rows (even stride) so they stay aligned.
    For the H pass the ACT (scalar) engine builds pv = [PAD, v, PAD] (copy
    of v at column offset 1, PAD columns memset on the DVE); then both H
    mins use only even offsets into v and pv:
        A[c]   = min(v[c], pv[c])    = min(v[c-1], v[c])
        out[c] = min(A[c], pv[c+2])  = min(v[c-1], v[c], v[c+1])
  - All mins on DVE (walrus rejects Pool-engine TensorTensor / TensorScalarPtr
    and DMA accum-min, verified on this toolchain); ACT does the shift copy
    and hosts the store HWDGE ring; SP ring hosts loads; Pool does the 4
    small bt DMAs.
  - Software-pipelined one tile deep (emit V(i), then H(i-1)) so the ACT
    copy latency hides behind the next tile's V pass.
  - A (the H intermediate) reuses the tmp tile (dead after the V pass; same
    engine in-order so no hazard) to keep SBUF at ~168 KiB/partition.
  - Cost model: DVE ~131k cycles @0.96 GHz = ~136 us/core/pass; HBM 32 MB
    @358 GB/s = ~90 us; ACT ~30 us. DVE-bound.
  - m>1 runs as m chained passes (DRAM ping-pong) inside one NEFF.
"""

import sys

sys.path.insert(0, "/opt/trn_rl_repo")

import numpy as np

import concourse.bass as bass
import concourse.tile as tile
from concourse import bacc, mybir

PAD = 1.0e9
BF16 = mybir.dt.bfloat16
MIN = mybir.AluOpType.min

CC = 1024  # column chunk width
RPP = 8  # image rows per partition

_cache = {}


def _emit_v(nc, pools, x_d, C, H, W, cc, rpp, s0, c0):
    """V phase for one tile: load, bt staging, vertical mins, pv build.
    Returns state for the H phase."""
    ppi = H // rpp
    ips = max(1, 128 // ppi)
    inp, bnd, vtm, vt, vsp, outp = pools
    R = rpp
    n_img = min(ips, C - s0)
    P = n_img * ppi

    t = inp.tile([128, R, cc], BF16)
    for i in range(n_img):
        src = x_d[s0 + i, :, c0 : c0 + cc].rearrange("(p r) w -> p r w", p=ppi)
        p0 = i * ppi
        nc.sync.dma_start(t[p0 : p0 + ppi], src)

    # boundary-row side tile: bt[p,0] = row below this partition's block
    # (t[p+1] row 0), bt[p,1] = row above (t[p-1] row R-1); at image edges
    # duplicate the edge row itself (min-idempotent clamp).
    bt = bnd.tile([128, 2, cc], BF16)
    for i in range(n_img):
        p0 = i * ppi
        pe = p0 + ppi - 1  # last partition of this image
        # the two big partition-shifted copies ride the SP/ACT HWDGE rings
        # (cheap FIFO slot right behind the load); the tiny edge clamps go
        # to Pool SWDGE so they don't occupy the rings.
        nc.sync.dma_start(bt[p0:pe, 0:1, :], t[p0 + 1 : pe + 1, 0:1, :])
        nc.gpsimd.dma_start(bt[pe : pe + 1, 0:1, :], t[pe : pe + 1, R - 1 : R, :])
        nc.scalar.dma_start(bt[p0 + 1 : pe + 1, 1:2, :], t[p0:pe, R - 1 : R, :])
        nc.gpsimd.dma_start(bt[p0 : p0 + 1, 1:2, :], t[p0 : p0 + 1, 0:1, :])

    # ---- V pass: v[r] = min(row r-1, r, r+1) ----
    # tmp[j] = min(row j, row j+1), j in [0, R-1); tmp[0] and tmp[R-2] double
    # as the boundary rows' first min stage. tmp has R rows: rows [0,R-1) are
    # the V intermediate, and the whole tile is recycled as the H
    # intermediate "A" after the V pass (DVE in-order => no hazard).
    v = vt.tile([128, R, cc], BF16)
    tmp = vtm.tile([128, R, cc], BF16)
    nc.vector.tensor_tensor(
        out=tmp[0:P, 0 : R - 1, :], in0=t[0:P, 0 : R - 1, :],
        in1=t[0:P, 1:R, :], op=MIN,
    )
    nc.vector.tensor_tensor(
        out=v[0:P, 1 : R - 1, :], in0=tmp[0:P, 0 : R - 2, :],
        in1=t[0:P, 2:R, :], op=MIN,
    )
    nc.vector.tensor_tensor(
        out=v[0:P, 0:1, :], in0=tmp[0:P, 0:1, :], in1=bt[0:P, 1:2, :], op=MIN,
    )
    nc.vector.tensor_tensor(
        out=v[0:P, R - 1 : R, :], in0=tmp[0:P, R - 2 : R - 1, :],
        in1=bt[0:P, 0:1, :], op=MIN,
    )

    # pv = [PAD | v | PAD]: PAD edge columns (image border, or the halo
    # column sourced from the neighbor chunk when cc < W) + ACT shift copy.
    # Engine writes are 16-bit granular (unlike sub-beat DMA RMW), so the
    # disjoint-column writes from DVE and ACT don't race.
    pv = vsp.tile([128, R, cc + 2], BF16)
    assert c0 == 0 and c0 + cc == W
    nc.vector.memset(pv[0:P, :, 0:1], PAD)
    nc.vector.memset(pv[0:P, :, cc + 1 : cc + 2], PAD)
    nc.scalar.copy(out=pv[0:P, :, 1 : cc + 1], in_=v[0:P])

    return (t, bt, tmp, v, pv, P, s0, c0)


def _emit_h(nc, pools, o_d, C, H, W, cc, rpp, state):
    """H phase for one tile: two aligned mins + store."""
    ppi = H // rpp
    inp, bnd, vtm, vt, vsp, outp = pools
    R = rpp
    t, bt, tmp, v, pv, P, s0, c0 = state
    n_img = P // ppi

    # ---- H pass: o[c] = min(v[c-1], v[c], v[c+1]) ----
    a = tmp[:, :, 0:cc]  # recycle tmp as the H intermediate
    nc.vector.tensor_tensor(
        out=a[0:P], in0=v[0:P], in1=pv[0:P, :, 0:cc], op=MIN,
    )
    ot = outp.tile([128, R, cc], BF16)
    nc.vector.tensor_tensor(
        out=ot[0:P], in0=a[0:P], in1=pv[0:P, :, 2 : cc + 2], op=MIN,
    )

    for i in range(n_img):
        dst = o_d[s0 + i, :, c0 : c0 + cc].rearrange("(p r) w -> p r w", p=ppi)
        p0 = i * ppi
        nc.scalar.dma_start(dst, ot[p0 : p0 + ppi, :, :])


def _emit_pass(nc, pools, x_d, o_d, C, H, W, cc, rpp):
    """Emit one full erosion pass x_d -> o_d, software-pipelined one tile."""
    ppi = H // rpp
    ips = max(1, 128 // ppi)
    pending = None
    for s0 in range(0, C, ips):
        for c0 in range(0, W, cc):
            st = _emit_v(nc, pools, x_d, C, H, W, cc, rpp, s0, c0)
            if pending is not None:
                _emit_h(nc, pools, o_d, C, H, W, cc, rpp, pending)
            pending = st
    _emit_h(nc, pools, o_d, C, H, W, cc, rpp, pending)


def build_erosion(C, H, W, cc=CC, rpp=RPP, reps=1, bufs=None):
    """Per-core Bass program: x [C,H,W] bf16 -> o [C,H,W] bf16, erosion^reps."""
    assert H % rpp == 0
    ppi = H // rpp
    assert ppi <= 128 and W % cc == 0
    assert cc == W, "chunked W needs halo columns from the neighbor chunk"

    nc = bacc.Bacc("TRN2", target_bir_lowering=False, debug=False, num_devices=1)
    x_d = nc.dram_tensor("x", [C, H, W], BF16, kind="ExternalInput").ap()
    o_d = nc.dram_tensor("o", [C, H, W], BF16, kind="ExternalOutput").ap()
    # ping-pong DRAM scratch for chained passes
    s_d = [
        nc.dram_tensor(f"scratch{i}", [C, H, W], BF16, kind="Internal").ap()
        for i in range(min(2, max(0, reps - 1)))
    ]

    def stage(i):
        src = x_d if i == 0 else s_d[(i - 1) % 2]
        dst = o_d if i == reps - 1 else s_d[i % 2]
        return src, dst

    bf = {"inp": 2, "bnd": 2, "vtm": 2, "vt": 2, "vsp": 2, "outp": 2}
    if bufs:
        bf.update(bufs)
    with tile.TileContext(nc) as tc:
        with (
            tc.tile_pool(name="inp", bufs=bf["inp"]) as inp,
            tc.tile_pool(name="bnd", bufs=bf["bnd"]) as bnd,
            tc.tile_pool(name="vtm", bufs=bf["vtm"]) as vtm,
            tc.tile_pool(name="vt", bufs=bf["vt"]) as vt,
            tc.tile_pool(name="vsp", bufs=bf["vsp"]) as vsp,
            tc.tile_pool(name="outp", bufs=bf["outp"]) as outp,
        ):
            pools = (inp, bnd, vtm, vt, vsp, outp)
            for i in range(reps):
                src, dst = stage(i)
                _emit_pass(nc, pools, src, dst, C, H, W, cc, rpp)
    nc.compile()
    return nc


def _get_program(C, H, W, reps=1):
    key = (C, H, W, reps)
    if key not in _cache:
        _cache[key] = build_erosion(C, H, W, cc=W, reps=reps)
    return _cache[key]


def _to_bf16(x):
    import ml_dtypes

    return np.asarray(x).astype(ml_dtypes.bfloat16)


def kernel(x, m):
    from concourse.bass_utils import run_bass_kernel_spmd

    m = int(np.asarray(m))
    x = np.ascontiguousarray(np.asarray(x), dtype=np.float32)
    B, C, H, W = x.shape
    if m <= 0:
        return x.copy()
    # erosion by a (2m+1)-square = m chained 3x3 erosion passes in one NEFF
    nc = _get_program(C, H, W, reps=m)
    n_cores = 8
    assert B == n_cores, f"expected batch {n_cores}, got {B}"
    xb = _to_bf16(x)
    in_maps = [{"x": xb[b]} for b in range(n_cores)]
    res = run_bass_kernel_spmd(nc, in_maps, core_ids=list(range(n_cores)))
    return np.stack(
        [r["o"].astype(np.float32) for r in res.results], axis=0
    )


if __name__ == "__main__":
    # small-scale CoreSim correctness check (no hardware needed)
    from concourse.bass_interp import CoreSim

    rng = np.random.default_rng(0)
    for C, H, W, cc, rpp in ((2, 128, 64, 64, 16), (1, 64, 64, 64, 8)):
        x = rng.standard_normal((C, H, W)).astype(np.float32)
        xb = _to_bf16(x)
        nc = build_erosion(C, H, W, cc=cc, rpp=rpp)
        sim = CoreSim(nc)
        sim.tensor("x")[:] = xb
        sim.simulate(check_with_hw=False)
        got = sim.tensor("o").astype(np.float32)
        xf = xb.astype(np.float32)
        xp = np.pad(xf, ((0, 0), (1, 1), (1, 1)), constant_values=PAD)
        exp = np.empty_like(xf)
        for i in range(H):
            for j in range(W):
                exp[:, i, j] = xp[:, i : i + 3, j : j + 3].min(axis=(1, 2))
        ok = np.array_equal(got, exp)
        print(f"CoreSim erosion C={C} H={H} W={W} cc={cc} rpp={rpp} ok: {ok}")
        assert ok


# revision 6
# speedup vs baseline: 1.0930x; 1.0930x over previous
"""Trainium2 Bass kernel for 3x3 (k=2m+1) morphological erosion (sliding-window
min) over [B, C, H, W] fp32, B=8 sharded across 8 NeuronCores (one batch per
core).

Numerics: min commutes with monotone rounding, so the device pipeline runs in
bf16 end-to-end (host converts fp32->bf16 on the way in, bf16->fp32 on the way
out). The device output equals bf16(exact fp32 min): max rel err 2^-9 ~ 0.2%,
well inside the 2e-2 gate. bf16 halves HBM traffic (32 MB/core/pass) and
doubles DVE tensor_tensor throughput (2x_1P packed mode).

Scheme (per core, shard = one batch of C=8 channel images, 1024x1024):
  - each partition holds RPP=8 consecutive image rows in its free dim; the
    vertical (row) min is a free-dim shifted tensor_tensor except at the 2
    per-partition boundary rows, whose missing neighbor rows are staged into
    a side tile "bt" via partition-shifted SBUF->SBUF DMA on the Pool SWDGE
    (DMA has no partition-alignment restriction; compute engines require
    start partition 0/32/64/96).
  - loads carry NO column halo: each partition's block is one fully aligned
    contiguous 16 KiB DRAM segment. (An earlier variant memset PAD halo
    columns beside the DMA'd data; sub-32B-beat writes from different queues
    RMW-race on hardware and corrupted ~1e-7 of elements at the borders.)
  - bf16 2x_1P packing requires step +-1 AND 4-byte-aligned operands. All
    V-pass operands shift by whole rows (even stride) so they stay aligned.
    For the H pass the ACT (scalar) engine builds pv = [PAD, v, PAD] (copy
    of v at column offset 1, PAD columns memset on the DVE); then both H
    mins use only even offsets into v and pv:
        A[c]   = min(v[c], pv[c])    = min(v[c-1], v[c])
        out[c] = min(A[c], pv[c+2])  = min(v[c-1], v[c], v[c+1])
  - All mins on DVE (walrus rejects Pool-engine TensorTensor / TensorScalarPtr
    and DMA accum-min, verified on this toolchain); ACT does the shift copy
    and hosts the store HWDGE ring; SP ring hosts loads; Pool does the 4
    small bt DMAs.
  - Software-pipelined one tile deep (emit V(i), then H(i-1)) so the ACT
    copy latency hides behind the next tile's V pass.
  - A (the H intermediate) reuses the tmp tile (dead after the V pass; same
    engine in-order so no hazard) to keep SBUF at ~168 KiB/partition.
  - Cost model: DVE ~131k cycles @0.96 GHz = ~136 us/core/pass; HBM 32 MB
    @358 GB/s = ~90 us; ACT ~30 us. DVE-bound.
  - m>1 runs as m chained passes (DRAM ping-pong) inside one NEFF.
"""

import sys

sys.path.insert(0, "/opt/trn_rl_repo")

import numpy as np

import concourse.bass as bass
import concourse.tile as tile
from concourse import bacc, mybir

PAD = 1.0e9
BF16 = mybir.dt.bfloat16
MIN = mybir.AluOpType.min

CC = 1024  # column chunk width
RPP = 8  # image rows per partition

_cache = {}


def _emit_v(nc, pools, x_d, C, H, W, cc, rpp, s0, c0):
    """V phase for one tile: load, bt staging, vertical mins, pv build.
    Returns state for the H phase."""
    ppi = H // rpp
    ips = max(1, 128 // ppi)
    inp, bnd, vtm, vt, vsp, outp = pools
    R = rpp
    n_img = min(ips, C - s0)
    P = n_img * ppi

    t = inp.tile([128, R, cc], BF16)
    for i in range(n_img):
        src = x_d[s0 + i, :, c0 : c0 + cc].rearrange("(p r) w -> p r w", p=ppi)
        p0 = i * ppi
        nc.sync.dma_start(t[p0 : p0 + ppi], src)

    # boundary-row side tile: bt[p,0] = row below this partition's block
    # (t[p+1] row 0), bt[p,1] = row above (t[p-1] row R-1); at image edges
    # duplicate the edge row itself (min-idempotent clamp).
    bt = bnd.tile([128, 2, cc], BF16)
    for i in range(n_img):
        p0 = i * ppi
        pe = p0 + ppi - 1  # last partition of this image
        src = x_d[s0 + i, :, c0 : c0 + cc].rearrange("(p r) w -> p r w", p=ppi)
        # boundary rows re-fetched straight from DRAM (regular pattern, no
        # dependency on the main load -> no HWDGE ring stall; +25% input
        # HBM, still far under the DVE bound); tiny edge clamps on SWDGE.
        nc.sync.dma_start(bt[p0:pe, 0:1, :], src[1:ppi, 0:1, :])
        nc.gpsimd.dma_start(bt[pe : pe + 1, 0:1, :], src[ppi - 1 : ppi, R - 1 : R, :])
        nc.sync.dma_start(bt[p0 + 1 : pe + 1, 1:2, :], src[0 : ppi - 1, R - 1 : R, :])
        nc.gpsimd.dma_start(bt[p0 : p0 + 1, 1:2, :], src[0:1, 0:1, :])

    # ---- V pass: v[r] = min(row r-1, r, r+1) ----
    # tmp[j] = min(row j, row j+1), j in [0, R-1); tmp[0] and tmp[R-2] double
    # as the boundary rows' first min stage. tmp has R rows: rows [0,R-1) are
    # the V intermediate, and the whole tile is recycled as the H
    # intermediate "A" after the V pass (DVE in-order => no hazard).
    v = vt.tile([128, R, cc], BF16)
    tmp = vtm.tile([128, R, cc], BF16)
    nc.vector.tensor_tensor(
        out=tmp[0:P, 0 : R - 1, :], in0=t[0:P, 0 : R - 1, :],
        in1=t[0:P, 1:R, :], op=MIN,
    )
    nc.vector.tensor_tensor(
        out=v[0:P, 1 : R - 1, :], in0=tmp[0:P, 0 : R - 2, :],
        in1=t[0:P, 2:R, :], op=MIN,
    )
    nc.vector.tensor_tensor(
        out=v[0:P, 0:1, :], in0=tmp[0:P, 0:1, :], in1=bt[0:P, 1:2, :], op=MIN,
    )
    nc.vector.tensor_tensor(
        out=v[0:P, R - 1 : R, :], in0=tmp[0:P, R - 2 : R - 1, :],
        in1=bt[0:P, 0:1, :], op=MIN,
    )

    # pv = [PAD | v | PAD]: PAD edge columns (image border, or the halo
    # column sourced from the neighbor chunk when cc < W) + ACT shift copy.
    # Engine writes are 16-bit granular (unlike sub-beat DMA RMW), so the
    # disjoint-column writes from DVE and ACT don't race.
    pv = vsp.tile([128, R, cc + 2], BF16)
    assert c0 == 0 and c0 + cc == W
    nc.vector.memset(pv[0:P, :, 0:1], PAD)
    nc.vector.memset(pv[0:P, :, cc + 1 : cc + 2], PAD)
    nc.scalar.copy(out=pv[0:P, :, 1 : cc + 1], in_=v[0:P])

    return (t, bt, tmp, v, pv, P, s0, c0)


def _emit_h(nc, pools, o_d, C, H, W, cc, rpp, state):
    """H phase for one tile: two aligned mins + store."""
    ppi = H // rpp
    inp, bnd, vtm, vt, vsp, outp = pools
    R = rpp
    t, bt, tmp, v, pv, P, s0, c0 = state
    n_img = P // ppi

    # ---- H pass: o[c] = min(v[c-1], v[c], v[c+1]) ----
    a = tmp[:, :, 0:cc]  # recycle tmp as the H intermediate
    nc.vector.tensor_tensor(
        out=a[0:P], in0=v[0:P], in1=pv[0:P, :, 0:cc], op=MIN,
    )
    ot = outp.tile([128, R, cc], BF16)
    nc.vector.tensor_tensor(
        out=ot[0:P], in0=a[0:P], in1=pv[0:P, :, 2 : cc + 2], op=MIN,
    )

    for i in range(n_img):
        dst = o_d[s0 + i, :, c0 : c0 + cc].rearrange("(p r) w -> p r w", p=ppi)
        p0 = i * ppi
        nc.scalar.dma_start(dst, ot[p0 : p0 + ppi, :, :])


def _emit_pass(nc, pools, x_d, o_d, C, H, W, cc, rpp):
    """Emit one full erosion pass x_d -> o_d, software-pipelined one tile."""
    ppi = H // rpp
    ips = max(1, 128 // ppi)
    pending = None
    for s0 in range(0, C, ips):
        for c0 in range(0, W, cc):
            st = _emit_v(nc, pools, x_d, C, H, W, cc, rpp, s0, c0)
            if pending is not None:
                _emit_h(nc, pools, o_d, C, H, W, cc, rpp, pending)
            pending = st
    _emit_h(nc, pools, o_d, C, H, W, cc, rpp, pending)


def build_erosion(C, H, W, cc=CC, rpp=RPP, reps=1, bufs=None):
    """Per-core Bass program: x [C,H,W] bf16 -> o [C,H,W] bf16, erosion^reps."""
    assert H % rpp == 0
    ppi = H // rpp
    assert ppi <= 128 and W % cc == 0
    assert cc == W, "chunked W needs halo columns from the neighbor chunk"

    nc = bacc.Bacc("TRN2", target_bir_lowering=False, debug=False, num_devices=1)
    x_d = nc.dram_tensor("x", [C, H, W], BF16, kind="ExternalInput").ap()
    o_d = nc.dram_tensor("o", [C, H, W], BF16, kind="ExternalOutput").ap()
    # ping-pong DRAM scratch for chained passes
    s_d = [
        nc.dram_tensor(f"scratch{i}", [C, H, W], BF16, kind="Internal").ap()
        for i in range(min(2, max(0, reps - 1)))
    ]

    def stage(i):
        src = x_d if i == 0 else s_d[(i - 1) % 2]
        dst = o_d if i == reps - 1 else s_d[i % 2]
        return src, dst

    bf = {"inp": 2, "bnd": 2, "vtm": 2, "vt": 2, "vsp": 2, "outp": 2}
    if bufs:
        bf.update(bufs)
    with tile.TileContext(nc) as tc:
        with (
            tc.tile_pool(name="inp", bufs=bf["inp"]) as inp,
            tc.tile_pool(name="bnd", bufs=bf["bnd"]) as bnd,
            tc.tile_pool(name="vtm", bufs=bf["vtm"]) as vtm,
            tc.tile_pool(name="vt", bufs=bf["vt"]) as vt,
            tc.tile_pool(name="vsp", bufs=bf["vsp"]) as vsp,
            tc.tile_pool(name="outp", bufs=bf["outp"]) as outp,
        ):
            pools = (inp, bnd, vtm, vt, vsp, outp)
            for i in range(reps):
                src, dst = stage(i)
                _emit_pass(nc, pools, src, dst, C, H, W, cc, rpp)
    nc.compile()
    return nc


def _get_program(C, H, W, reps=1):
    key = (C, H, W, reps)
    if key not in _cache:
        _cache[key] = build_erosion(C, H, W, cc=W, reps=reps)
    return _cache[key]


def _to_bf16(x):
    import ml_dtypes

    return np.asarray(x).astype(ml_dtypes.bfloat16)


def kernel(x, m):
    from concourse.bass_utils import run_bass_kernel_spmd

    m = int(np.asarray(m))
    x = np.ascontiguousarray(np.asarray(x), dtype=np.float32)
    B, C, H, W = x.shape
    if m <= 0:
        return x.copy()
    # erosion by a (2m+1)-square = m chained 3x3 erosion passes in one NEFF
    nc = _get_program(C, H, W, reps=m)
    n_cores = 8
    assert B == n_cores, f"expected batch {n_cores}, got {B}"
    xb = _to_bf16(x)
    in_maps = [{"x": xb[b]} for b in range(n_cores)]
    res = run_bass_kernel_spmd(nc, in_maps, core_ids=list(range(n_cores)))
    return np.stack(
        [r["o"].astype(np.float32) for r in res.results], axis=0
    )


if __name__ == "__main__":
    # small-scale CoreSim correctness check (no hardware needed)
    from concourse.bass_interp import CoreSim

    rng = np.random.default_rng(0)
    for C, H, W, cc, rpp in ((2, 128, 64, 64, 16), (1, 64, 64, 64, 8)):
        x = rng.standard_normal((C, H, W)).astype(np.float32)
        xb = _to_bf16(x)
        nc = build_erosion(C, H, W, cc=cc, rpp=rpp)
        sim = CoreSim(nc)
        sim.tensor("x")[:] = xb
        sim.simulate(check_with_hw=False)
        got = sim.tensor("o").astype(np.float32)
        xf = xb.astype(np.float32)
        xp = np.pad(xf, ((0, 0), (1, 1), (1, 1)), constant_values=PAD)
        exp = np.empty_like(xf)
        for i in range(H):
            for j in range(W):
                exp[:, i, j] = xp[:, i : i + 3, j : j + 3].min(axis=(1, 2))
        ok = np.array_equal(got, exp)
        print(f"CoreSim erosion C={C} H={H} W={W} cc={cc} rpp={rpp} ok: {ok}")
        assert ok


# revision 8
# speedup vs baseline: 2.3533x; 2.1530x over previous
"""Trainium2 Bass kernel for 3x3 (k=2m+1) morphological erosion (sliding-window
min) over [B, C, H, W] fp32, B=8 sharded across 8 NeuronCores (one batch per
core).

Numerics: min commutes with monotone rounding, so the device pipeline runs in
bf16 end-to-end (host converts fp32->bf16 on the way in, bf16->fp32 on the way
out). The device output equals bf16(exact fp32 min): max rel err 2^-9 ~ 0.2%,
well inside the 2e-2 gate. bf16 halves HBM traffic (32 MB/core/pass) and
doubles DVE throughput (packed 2x modes).

Scheme (per core, shard = one batch of C=8 channel images, 1024x1024):
  - each partition holds RPP=8 consecutive image rows in its free dim; the
    vertical (row) min is a free-dim shifted tensor_tensor except at the 2
    per-partition boundary rows, whose missing neighbor rows are staged into
    a side tile "bt" via partition-shifted SBUF->SBUF DMA on the Pool SWDGE
    (DMA has no partition-alignment restriction; compute engines require
    start partition 0/32/64/96; HWDGE-ring variants of these copies measured
    ~2x slower end-to-end: ring FIFO + descriptor pressure).
  - loads carry NO column halo: each partition's block is one fully aligned
    contiguous 16 KiB DRAM segment. (An earlier variant memset PAD halo
    columns beside the DMA'd data; sub-32B-beat writes from different queues
    RMW-race on hardware and corrupted ~1e-7 of elements at the borders.)
  - the ACT (scalar) engine builds pv = [PAD, v, PAD] (copy of the V-pass
    result at column offset 1; PAD columns memset on the DVE; engine writes
    are 16-bit granular so the disjoint-column writes don't race).
  - the whole H pass is ONE custom DVE instruction (MIN3S_ANT, defined
    below): out[c] = min(pv[c], pv[c+1], pv[c+2]) at 2 results/cycle in the
    packed 2X_1PORT mode, replacing two tensor_tensor mins.
  - All mins on DVE (walrus rejects Pool-engine TensorTensor /
    TensorScalarPtr and DMA accum-min on this toolchain); ACT does the shift
    copy and hosts the store HWDGE ring; SP ring hosts loads; Pool does the
    4 small bt DMAs.
  - Software-pipelined one tile deep (emit V(i), then H(i-1)) so the ACT
    copy latency hides behind the next tile's V pass.
  - Cost model: DVE ~99k cycles @0.96 GHz = ~103 us/core/pass; HBM 32 MB
    @358 GB/s = ~90 us; ACT ~80 us. DVE-bound.
  - m>1 runs as m chained passes (DRAM ping-pong) inside one NEFF.
"""

import sys

sys.path.insert(0, "/opt/trn_rl_repo")

import numpy as np

import concourse.bass as bass
import concourse.tile as tile
from concourse import bacc, bass_isa, mybir
from concourse import dve_ops as _dve_ops
from concourse.dve_spec import Spec, Src0, Src1, minn
from concourse.dve_uop import (
    AluInp,
    AluOp,
    DelayInp,
    DveOpSpec,
    InpSel,
    OutPath,
    OutSel,
    Trigger,
    UopConfig,
)

PAD = 1.0e9
BF16 = mybir.dt.bfloat16
MIN = mybir.AluOpType.min

CC = 1024  # column chunk width
RPP = 8  # image rows per partition

_cache = {}


# --------------------------------------------------------------------------- #
# Custom DVE op MIN3S_ANT: packed sliding min-of-3 for the erosion H pass.
#
# Semantics (bf16, 2X_1PORT packed mode, call sites pass in1 = in0 + 2 elems):
#     out[2j]   = min(in0[2j], in0[2j+1], in1[2j])
#     out[2j+1] = min(in0[2j+1], in1[2j], in1[2j+1])
# With in0 = pv[c], in1 = pv[c+2] this is out[c] = min3(pv[c..c+2]): the whole
# horizontal pass in ONE instruction at 2 results/cycle.
#
# The 2x uop program reads the packed pair lanes SRC_0/SRC_0_HI/SRC_1/SRC_1_HI
# (4 of the 7 crossbar lanes), computes m = min(S0H, S1) once, then
# lo = min(S0, m), hi = min(m, S1H), writing the packed pair via WR0_LO/WR0_HI
# -- mirroring the stock TENSOR_TENSOR 2X_1PORT table entry (decoded from
# neuronxcc's default_*.bin tables).
#
# The REGULAR (1x) fallback computes min(in0, in1), which is NOT min3 -- call
# sites must satisfy the 2x auto-detect conditions (16-bit dtype, innermost
# step 1, 4B-aligned operands, even run lengths) so the engine always engages
# slot +1 (gated by perf_max=1 in byte 36). A silent 1x fallback shows up as a
# gross mismatch in every correctness test (verified bit-exact on HW).
# --------------------------------------------------------------------------- #

_MIN3_NAME = "MIN3S_ANT"


def _min3_reference(in0, in1, c0, c1, c2):
    a = np.asarray(in0, np.float32)
    b = np.asarray(in1, np.float32)
    assert a.shape[-1] % 2 == 0
    e0, o0 = a[..., 0::2], a[..., 1::2]
    e1, o1 = b[..., 0::2], b[..., 1::2]
    out = np.empty_like(a)
    out[..., 0::2] = np.minimum(np.minimum(e0, o0), e1)
    out[..., 1::2] = np.minimum(np.minimum(o0, e1), o1)
    return out


def _min3_build_uops():
    u1 = UopConfig()
    u1.enable_input(InpSel.SRC_0, 0)
    u1.enable_input(InpSel.SRC_1, 1)
    u1.require_inp0 = u1.require_inp1 = 1
    u1.trigger = (Trigger.SRC_TENSOR_DONE, Trigger.NONE, Trigger.NONE)
    u1.datapath_config[0].enable_alu(
        AluOp.MIN, AluInp.PREV_ALU_OUT, AluInp.PREV_DELAY_0
    )
    for b in range(1, 8):
        u1.datapath_config[b].pass_through_alu()
    u1.enable_output(OutSel.ALU_OUT, OutPath.WR0_LO)

    u2 = UopConfig()
    u2.enable_input(InpSel.SRC_0, 0)  # lane0: S0  = pv[2j]
    u2.enable_input(InpSel.SRC_1, 1)  # lane1: S1  = pv[2j+2]
    u2.enable_input(InpSel.SRC_0_HI, 2)  # lane2: S0H = pv[2j+1]
    u2.enable_input(InpSel.SRC_1_HI, 3)  # lane3: S1H = pv[2j+3]
    u2.require_inp0 = u2.require_inp1 = 1
    u2.trigger = (Trigger.SRC_TENSOR_DONE, Trigger.NONE, Trigger.NONE)
    dp = u2.datapath_config
    dp[0].enable_alu(AluOp.MIN, AluInp.PREV_DELAY_1, AluInp.PREV_DELAY_0)
    dp[0].enable_delay_from_src(DelayInp.PREV_ALU_OUT, 0)  # chain0 <- lane0 (S0)
    dp[0].pass_through_delay(2)  # chain2 <- lane3 (S1H)
    dp[1].enable_alu(AluOp.MIN, AluInp.PREV_ALU_OUT, AluInp.PREV_DELAY_0)
    dp[1].enable_delay_from_src(DelayInp.PREV_ALU_OUT, 1)  # chain1 <- m
    dp[1].pass_through_delay(2)
    dp[2].enable_alu(AluOp.MIN, AluInp.PREV_DELAY_1, AluInp.PREV_DELAY_2)
    dp[2].enable_delay_from_src(DelayInp.PREV_ALU_OUT, 0)  # chain0 <- lo
    for b in range(3, 8):
        dp[b].pass_through_alu()
        dp[b].pass_through_delay(0)
    u2.enable_output(OutSel.DELAY_0, OutPath.WR0_LO)  # lo -> low half
    u2.enable_output(OutSel.ALU_OUT, OutPath.WR0_HI)  # hi -> high half
    return [u1], [u2]


class _Min3Op:
    """Duck-types dve_ops.DveOp for the OPS/table-gen/CoreSim registries."""

    name = _MIN3_NAME
    subdim = False
    perf_en = {"v3": True}

    def __init__(self, row):
        self.row = row
        uops, uops_2x = _min3_build_uops()
        self._spec = DveOpSpec(
            name=_MIN3_NAME,
            opcode=row,
            uops=uops,
            uops_2x=uops_2x,
            perf_max=1,
            rd1_en=True,
        )
        # CoreSim executes custom ops via CUSTOM_DVE_SPECS[name].reference;
        # the body is a placeholder (never lowered -- compile() returns the
        # hand-built uops above).
        self.spec = Spec(body=minn(Src0, Src1), reference=_min3_reference)

    def compile(self, ver):
        assert ver == "v3", f"MIN3S_ANT is TRN2-only (got {ver})"
        return self._spec


_min3_op = None


def _get_min3_op():
    global _min3_op
    if _min3_op is None:
        if _MIN3_NAME in _dve_ops._SUB_OPCODE_FOR_NAME:
            raise RuntimeError(f"{_MIN3_NAME} already registered")
        row = _dve_ops._CUSTOM_DVE_ROW_BASE + len(_dve_ops.OPS)
        assert row < 0x20, "no free byte-36 rows left"
        _min3_op = _Min3Op(row)
        _dve_ops.OPS.append(_min3_op)
        _dve_ops.CUSTOM_DVE_SPECS[_MIN3_NAME] = _min3_op.spec
        _dve_ops._SUB_OPCODE_FOR_NAME[_MIN3_NAME] = row
    return _min3_op


def _emit_min3(nc, out, in0, in1):
    op = _get_min3_op()
    ve = nc.vector
    for ap in (out, in0, in1):
        assert ap.dtype == mybir.dt.bfloat16
        assert ap.shape[-1] % 2 == 0
    if op.name not in nc.m.ant_custom_dve_ops:
        nc.m.ant_custom_dve_ops = sorted({*nc.m.ant_custom_dve_ops, op.name})
    shape = bass_isa.CustomDveShape.STT  # full-rank src1
    isa_opcode = nc.isa.Opcode[
        f"NEURON_ISA_TPB_OPCODE_CUSTOM_DVE_ANT_{shape.slot()}"
    ].value
    zero = mybir.ImmediateValue(dtype=mybir.dt.float32, value=0.0)
    return ve.add_instruction(
        bass_isa.InstCustomDveAnt(
            name=nc.get_next_instruction_name(),
            op_name=op.name,
            rd1_en=True,
            subdim=0,
            imm2=0.0,
            shape=shape,
            row=op.row,
            perf_max=1,
            isa_opcode=isa_opcode,
            ins=[
                ve.lower_ap(in0, for_isa=True),
                ve.lower_ap(in1, for_isa=True),
                zero,
                zero,
            ],
            outs=[ve.lower_ap(out, for_isa=True)],
        )
    )


# --------------------------------------------------------------------------- #
# The erosion kernel
# --------------------------------------------------------------------------- #


def _emit_v(nc, pools, x_d, C, H, W, cc, rpp, s0, c0):
    """V phase for one tile: load, bt staging, vertical mins, pv build.
    Returns state for the H phase."""
    ppi = H // rpp
    ips = max(1, 128 // ppi)
    inp, bnd, vtm, vt, vsp, outp = pools
    R = rpp
    n_img = min(ips, C - s0)
    P = n_img * ppi

    t = inp.tile([128, R, cc], BF16)
    for i in range(n_img):
        src = x_d[s0 + i, :, c0 : c0 + cc].rearrange("(p r) w -> p r w", p=ppi)
        p0 = i * ppi
        nc.sync.dma_start(t[p0 : p0 + ppi], src)

    # boundary-row side tile: bt[p,0] = row below this partition's block
    # (t[p+1] row 0), bt[p,1] = row above (t[p-1] row R-1); at image edges
    # duplicate the edge row itself (min-idempotent clamp).
    bt = bnd.tile([128, 2, cc], BF16)
    for i in range(n_img):
        p0 = i * ppi
        pe = p0 + ppi - 1  # last partition of this image
        nc.gpsimd.dma_start(bt[p0:pe, 0:1, :], t[p0 + 1 : pe + 1, 0:1, :])
        nc.gpsimd.dma_start(bt[pe : pe + 1, 0:1, :], t[pe : pe + 1, R - 1 : R, :])
        nc.gpsimd.dma_start(bt[p0 + 1 : pe + 1, 1:2, :], t[p0:pe, R - 1 : R, :])
        nc.gpsimd.dma_start(bt[p0 : p0 + 1, 1:2, :], t[p0 : p0 + 1, 0:1, :])

    # ---- V pass: v[r] = min(row r-1, r, r+1) ----
    # tmp[j] = min(row j, row j+1), j in [0, R-1); tmp[0] and tmp[R-2] double
    # as the boundary rows' first min stage.
    v = vt.tile([128, R, cc], BF16)
    tmp = vtm.tile([128, R - 1, cc], BF16)
    nc.vector.tensor_tensor(
        out=tmp[0:P], in0=t[0:P, 0 : R - 1, :], in1=t[0:P, 1:R, :], op=MIN,
    )
    nc.vector.tensor_tensor(
        out=v[0:P, 1 : R - 1, :], in0=tmp[0:P, 0 : R - 2, :],
        in1=t[0:P, 2:R, :], op=MIN,
    )
    nc.vector.tensor_tensor(
        out=v[0:P, 0:1, :], in0=tmp[0:P, 0:1, :], in1=bt[0:P, 1:2, :], op=MIN,
    )
    nc.vector.tensor_tensor(
        out=v[0:P, R - 1 : R, :], in0=tmp[0:P, R - 2 : R - 1, :],
        in1=bt[0:P, 0:1, :], op=MIN,
    )

    # pv = [PAD | v | PAD]: PAD edge columns + ACT shift copy. Engine writes
    # are 16-bit granular (unlike sub-beat DMA RMW), so the disjoint-column
    # writes from DVE and ACT don't race.
    pv = vsp.tile([128, R, cc + 2], BF16)
    assert c0 == 0 and c0 + cc == W
    nc.vector.memset(pv[0:P, :, 0:1], PAD)
    nc.vector.memset(pv[0:P, :, cc + 1 : cc + 2], PAD)
    nc.scalar.copy(out=pv[0:P, :, 1 : cc + 1], in_=v[0:P])

    return (t, bt, tmp, v, pv, P, s0, c0)


def _emit_h(nc, pools, o_d, C, H, W, cc, rpp, state):
    """H phase for one tile: one packed min3 + store."""
    ppi = H // rpp
    inp, bnd, vtm, vt, vsp, outp = pools
    R = rpp
    t, bt, tmp, v, pv, P, s0, c0 = state
    n_img = P // ppi

    # ---- H pass: o[c] = min(pv[c], pv[c+1], pv[c+2]) in one instruction ----
    ot = outp.tile([128, R, cc], BF16)
    _emit_min3(nc, ot[0:P], pv[0:P, :, 0:cc], pv[0:P, :, 2 : cc + 2])

    for i in range(n_img):
        dst = o_d[s0 + i, :, c0 : c0 + cc].rearrange("(p r) w -> p r w", p=ppi)
        p0 = i * ppi
        nc.scalar.dma_start(dst, ot[p0 : p0 + ppi, :, :])


def _emit_pass(nc, pools, x_d, o_d, C, H, W, cc, rpp):
    """Emit one full erosion pass x_d -> o_d, software-pipelined one tile."""
    ppi = H // rpp
    ips = max(1, 128 // ppi)
    pending = None
    for s0 in range(0, C, ips):
        for c0 in range(0, W, cc):
            st = _emit_v(nc, pools, x_d, C, H, W, cc, rpp, s0, c0)
            if pending is not None:
                _emit_h(nc, pools, o_d, C, H, W, cc, rpp, pending)
            pending = st
    _emit_h(nc, pools, o_d, C, H, W, cc, rpp, pending)


def build_erosion(C, H, W, cc=CC, rpp=RPP, reps=1, bufs=None):
    """Per-core Bass program: x [C,H,W] bf16 -> o [C,H,W] bf16, erosion^reps."""
    assert H % rpp == 0
    ppi = H // rpp
    assert ppi <= 128 and W % cc == 0
    assert cc == W, "chunked W needs halo columns from the neighbor chunk"
    assert cc % 2 == 0, "min3 packed mode needs even run lengths"

    nc = bacc.Bacc("TRN2", target_bir_lowering=False, debug=False, num_devices=1)
    x_d = nc.dram_tensor("x", [C, H, W], BF16, kind="ExternalInput").ap()
    o_d = nc.dram_tensor("o", [C, H, W], BF16, kind="ExternalOutput").ap()
    # ping-pong DRAM scratch for chained passes
    s_d = [
        nc.dram_tensor(f"scratch{i}", [C, H, W], BF16, kind="Internal").ap()
        for i in range(min(2, max(0, reps - 1)))
    ]

    def stage(i):
        src = x_d if i == 0 else s_d[(i - 1) % 2]
        dst = o_d if i == reps - 1 else s_d[i % 2]
        return src, dst

    bf = {"inp": 2, "bnd": 2, "vtm": 2, "vt": 2, "vsp": 2, "outp": 2}
    if bufs:
        bf.update(bufs)
    with tile.TileContext(nc) as tc:
        with (
            tc.tile_pool(name="inp", bufs=bf["inp"]) as inp,
            tc.tile_pool(name="bnd", bufs=bf["bnd"]) as bnd,
            tc.tile_pool(name="vtm", bufs=bf["vtm"]) as vtm,
            tc.tile_pool(name="vt", bufs=bf["vt"]) as vt,
            tc.tile_pool(name="vsp", bufs=bf["vsp"]) as vsp,
            tc.tile_pool(name="outp", bufs=bf["outp"]) as outp,
        ):
            pools = (inp, bnd, vtm, vt, vsp, outp)
            for i in range(reps):
                src, dst = stage(i)
                _emit_pass(nc, pools, src, dst, C, H, W, cc, rpp)
    nc.compile()
    return nc


def _get_program(C, H, W, reps=1):
    key = (C, H, W, reps)
    if key not in _cache:
        _cache[key] = build_erosion(C, H, W, cc=W, reps=reps)
    return _cache[key]


def _to_bf16(x):
    import ml_dtypes

    return np.asarray(x).astype(ml_dtypes.bfloat16)


def kernel(x, m):
    from concourse.bass_utils import run_bass_kernel_spmd

    m = int(np.asarray(m))
    x = np.ascontiguousarray(np.asarray(x), dtype=np.float32)
    B, C, H, W = x.shape
    if m <= 0:
        return x.copy()
    # erosion by a (2m+1)-square = m chained 3x3 erosion passes in one NEFF
    nc = _get_program(C, H, W, reps=m)
    n_cores = 8
    assert B == n_cores, f"expected batch {n_cores}, got {B}"
    xb = _to_bf16(x)
    in_maps = [{"x": xb[b]} for b in range(n_cores)]
    res = run_bass_kernel_spmd(nc, in_maps, core_ids=list(range(n_cores)))
    return np.stack(
        [r["o"].astype(np.float32) for r in res.results], axis=0
    )


if __name__ == "__main__":
    # small-scale CoreSim correctness check (no hardware needed)
    from concourse.bass_interp import CoreSim

    rng = np.random.default_rng(0)
    for C, H, W, cc, rpp in ((2, 128, 64, 64, 16), (1, 64, 64, 64, 8)):
        x = rng.standard_normal((C, H, W)).astype(np.float32)
        xb = _to_bf16(x)
        nc = build_erosion(C, H, W, cc=cc, rpp=rpp)
        sim = CoreSim(nc)
        sim.tensor("x")[:] = xb
        sim.simulate(check_with_hw=False)
        got = sim.tensor("o").astype(np.float32)
        xf = xb.astype(np.float32)
        xp = np.pad(xf, ((0, 0), (1, 1), (1, 1)), constant_values=PAD)
        exp = np.empty_like(xf)
        for i in range(H):
            for j in range(W):
                exp[:, i, j] = xp[:, i : i + 3, j : j + 3].min(axis=(1, 2))
        ok = np.array_equal(got, exp)
        print(f"CoreSim erosion C={C} H={H} W={W} cc={cc} rpp={rpp} ok: {ok}")
        assert ok
